# revision 20
# baseline (speedup 1.0000x reference)
"""Trainium2 Bass kernel for nn_BinaryConv2d (binary conv + batchnorm + sign).

Computation (reference):
    alpha = mean(|w|, axis=(1,2,3))            # per out-channel scale
    y     = conv2d(x, sign(w)*alpha, pad=1)    # NCHW, 3x3, stride 1
    y     = batchnorm(y, batch stats over (N,H,W), eps=1e-5, gamma, beta)
    out   = clip(sign(y), -1, 1)

Distribution: batch dim (64) sharded 8-way across NeuronCores; BN stats
all-reduced on-device (per-channel sum + sumsq, 32 floats).

Device algorithm per core (8 images):
  The 3x3 conv is computed as 3 matmuls per 6-output-row window:
    stationary W_j [K=128, M=96], K=(x-row-block b in 0..7, in-ch i),
    M=(out-row r in 0..5, out-ch o); W_j[b*16+i, r*16+o]=sign(w)[o,i,b-r,j].
    Moving operand = host-prepared im2col-block tensor xprep
    [img, b, i, win, W+2] (zero-padded rows/cols baked in by host), tap j
    streams cols j..j+W-1.  All 3 taps accumulate into one PSUM tile
    [96, W] which then holds final (unscaled) conv output for 6 rows.
  alpha is folded into the BN affine: out = Sign(scale_o * y_psum + bias_o)
  with scale = gamma*alpha*rsqrt(alpha^2*var+eps),
  bias = beta - scale*alpha*mean — computed on-device from global stats.
  Pass 1: bf16(x_hi) conv -> bn_stats per window -> bn_aggr -> per-channel
          reduce (tiny fp32 matmul vs a selector) -> AllReduce([16,2]).
  Pass 2: exact conv via x_hi+x_lo bf16 split (sign weights are exact in
          bf16, so hi+lo matmuls reproduce fp32 x to ~2^-18) -> one ScalarE
          Sign activation per window -> DMA out.
"""

import numpy as np
import ml_dtypes

import concourse.bass as bass
import concourse.tile as tile
from concourse import bacc, mybir

BF16 = mybir.dt.bfloat16
F32 = mybir.dt.float32
BN_EPS = 1e-5

# Full-size problem config
FULL = dict(n_img=8, H=256, W=256, n_cores=8)
C = 16          # channels (in == out)
WIN = 6         # output rows per window
B = WIN + 2     # x-row blocks per window
K = B * C       # 128 contraction
M = WIN * C     # 96 psum partitions


def _cfg(n_img, H, W):
    NW = -(-H // WIN)              # windows per image
    PADW = W + 2
    return NW, PADW


def build_nc(n_img, H, W, n_cores, kwin=4, debug=False, dbg_out=False):
    """Build the SPMD bass graph (same NEFF on every core)."""
    NW, PADW = _cfg(n_img, H, W)
    rem = H - (NW - 1) * WIN       # valid out rows in last window
    cnt = float(n_img * NW * W)    # bn_stats element count per partition
    n_tot = float(n_cores * n_img * H * W)  # true per-channel count

    nc = bacc.Bacc("TRN2", target_bir_lowering=False, debug=debug,
                   num_devices=n_cores)

    xh = nc.dram_tensor("xh", [n_img, B, C, NW, PADW], BF16, kind="ExternalInput")
    xl = nc.dram_tensor("xl", [n_img, B, C, NW, PADW], BF16, kind="ExternalInput")
    wj = nc.dram_tensor("wj", [3, K, M], BF16, kind="ExternalInput")
    sel = nc.dram_tensor("sel", [M, C], F32, kind="ExternalInput")
    cvec = nc.dram_tensor("cvec", [C, 4], F32, kind="ExternalInput")  # A,A2,GA,BETA
    # device-friendly layout; host transposes/slices to [n_img, C, H, W]
    out = nc.dram_tensor("out", [n_img, NW, WIN, C, W], F32,
                         kind="ExternalOutput")
    if dbg_out:
        dbg_sb = nc.dram_tensor("dbg_sb", [C, 2], F32, kind="ExternalOutput")
        dbg_g = nc.dram_tensor("dbg_g", [C, 2], F32, kind="ExternalOutput")
        dbg_agg = nc.dram_tensor("dbg_agg", [M, 2], F32, kind="ExternalOutput")
        dbg_st = nc.dram_tensor("dbg_st", [M, n_img * NW * 6], F32,
                                kind="ExternalOutput")

    xh_ap, xl_ap, out_ap = xh.ap(), xl.ap(), out.ap()

    with tile.TileContext(nc) as tc:
        with (
            tc.tile_pool(name="consts", bufs=1) as consts,
            tc.tile_pool(name="xin", bufs=3) as xin,
            tc.tile_pool(name="xin2", bufs=3) as xin2,
            tc.tile_pool(name="osb", bufs=3) as osbp,
            tc.tile_pool(name="psum", bufs=7, space="PSUM") as psum,
            tc.tile_pool(name="psc", bufs=1, space="PSUM") as pscp,
            tc.tile_pool(name="dram", bufs=1, space="DRAM") as dram,
        ):
            # --- constants ---
            w_sb = consts.tile([K, 3 * M], BF16)
            nc.sync.dma_start(
                w_sb[:].rearrange("k (j m) -> k j m", j=3),
                wj.ap().transpose([1, 0, 2]))
            sel_sb = consts.tile([M, C], F32)
            nc.sync.dma_start(sel_sb[:], sel.ap())
            cv = consts.tile([C, 4], F32)
            nc.sync.dma_start(cv[:], cvec.ap())

            stats_buf = consts.tile([M, n_img * NW * 6], F32)
            if rem < WIN:
                # garbage out-rows of each image's last window would pollute
                # stats (row H touches real x row H-1); zero their slots and
                # bn_stats only the valid partitions there.
                nc.vector.memset(stats_buf[rem * C:M, :], 0.0)

            def conv_mms(ps, xt, dw, first, last):
                for j in range(3):
                    nc.tensor.matmul(
                        ps[:], w_sb[:, j * M:(j + 1) * M],
                        xt[:, dw, j:j + W],
                        start=(first and j == 0), stop=(last and j == 2))

            # ---------------- pass 1: statistics ----------------
            for n in range(n_img):
                for w0 in range(0, NW, kwin):
                    kw = min(kwin, NW - w0)
                    xt = xin.tile([K, kwin, PADW], BF16, tag="xh")
                    nc.sync.dma_start(
                        xt[:, 0:kw, :],
                        xh_ap[n, :, :, w0:w0 + kw, :].rearrange(
                            "b i k c -> (b i) k c"))
                    for dw in range(kw):
                        ps = psum.tile([M, W], F32, tag="ps")
                        conv_mms(ps, xt, dw, True, True)
                        g_idx = n * NW + w0 + dw
                        mp = rem * C if (w0 + dw == NW - 1 and rem < WIN) else M
                        nc.vector.bn_stats(
                            stats_buf[0:mp, g_idx * 6:(g_idx + 1) * 6],
                            ps[0:mp, :])

            # ---------------- stats reduce + allreduce ----------------
            # bn_stats triples are (count, mean, count*var) x (even, odd).
            # All live slots have count = W/2; zeroed slots contribute 0 to
            # every term below, so a constant count works for all of them.
            #   S_p = (W/2) * sum(means);  Q_p = sum(M2) + (W/2)*sum(means^2)
            ns2 = n_img * NW * 2
            sb3 = stats_buf[:].rearrange("p (s t) -> p s t", t=3)
            means = sb3[:, :, 1]
            m2s = sb3[:, :, 2]
            smean = consts.tile([M, 1], F32)
            qa = consts.tile([M, 1], F32)
            qb = consts.tile([M, 1], F32)
            tmpm = consts.tile([M, ns2], F32)
            nc.vector.reduce_sum(smean[:], means, axis=mybir.AxisListType.X)
            nc.vector.reduce_sum(qa[:], m2s, axis=mybir.AxisListType.X)
            nc.vector.tensor_mul(tmpm[:], means, means)
            nc.vector.reduce_sum(qb[:], tmpm[:], axis=mybir.AxisListType.X)
            sums = consts.tile([M, 2], F32)          # (S, Q) per partition
            half = float(W // 2)
            nc.vector.tensor_scalar_mul(sums[:, 0:1], smean[:], half)
            nc.vector.tensor_scalar_mul(qb[:], qb[:], half)
            nc.vector.tensor_add(sums[:, 1:2], qa[:], qb[:])

            psc = pscp.tile([C, 2], F32)
            nc.tensor.matmul(psc[:], sel_sb[:], sums[:], start=True, stop=True)
            ccin_sb = consts.tile([C, 2], F32)
            nc.vector.tensor_copy(ccin_sb[:], psc[:])

            cc_in = dram.tile([C, 2], F32)
            cc_out = dram.tile([C, 2], F32,
                               addr_space="Shared" if n_cores > 4 else "Local")
            nc.sync.dma_start(cc_in[:], ccin_sb[:])
            nc.gpsimd.collective_compute(
                "AllReduce", mybir.AluOpType.add,
                replica_groups=[list(range(n_cores))],
                ins=[cc_in[:].opt()], outs=[cc_out[:].opt()])
            gsb = consts.tile([C, 2], F32)
            nc.sync.dma_start(gsb[:], cc_out[:])

            # scale = GA*rsqrt(A2*var+eps); bias = BETA - scale*A*mean
            m_ = consts.tile([C, 1], F32)
            e2 = consts.tile([C, 1], F32)
            t0 = consts.tile([C, 1], F32)
            var_ = consts.tile([C, 1], F32)
            rec = consts.tile([C, 1], F32)
            r_ = consts.tile([C, 1], F32)
            t1 = consts.tile([C, 1], F32)
            t2 = consts.tile([C, 1], F32)
            sb16 = consts.tile([C, 2], F32)
            nc.vector.tensor_scalar_mul(m_[:], gsb[:, 0:1], 1.0 / n_tot)
            nc.vector.tensor_scalar_mul(e2[:], gsb[:, 1:2], 1.0 / n_tot)
            nc.vector.tensor_mul(t0[:], m_[:], m_[:])
            nc.vector.tensor_sub(var_[:], e2[:], t0[:])
            nc.vector.tensor_mul(var_[:], var_[:], cv[:, 1:2])
            nc.vector.tensor_scalar_add(var_[:], var_[:], BN_EPS)
            nc.vector.reciprocal(rec[:], var_[:])
            nc.scalar.sqrt(r_[:], rec[:])
            nc.vector.tensor_mul(sb16[:, 0:1], cv[:, 2:3], r_[:])
            nc.vector.tensor_mul(t2[:], sb16[:, 0:1], m_[:])
            nc.vector.tensor_sub(sb16[:, 1:2], cv[:, 3:4], t2[:])
            sb96 = consts.tile([M, 2], F32)
            for r in range(WIN):
                nc.sync.dma_start(sb96[r * C:(r + 1) * C, :], sb16[:])
            if dbg_out:
                nc.sync.dma_start(dbg_sb.ap(), sb16[:])
                nc.sync.dma_start(dbg_g.ap(), gsb[:])
                nc.sync.dma_start(dbg_agg.ap(), sums[:])
                nc.sync.dma_start(dbg_st.ap(), stats_buf[:])

            # ---------------- pass 2: output ----------------
            for n in range(n_img):
                for w0 in range(0, NW, kwin):
                    kw = min(kwin, NW - w0)
                    xt = xin.tile([K, kwin, PADW], BF16, tag="xh")
                    nc.sync.dma_start(
                        xt[:, 0:kw, :],
                        xh_ap[n, :, :, w0:w0 + kw, :].rearrange(
                            "b i k c -> (b i) k c"))
                    xt2 = xin2.tile([K, kwin, PADW], BF16, tag="xl")
                    nc.sync.dma_start(
                        xt2[:, 0:kw, :],
                        xl_ap[n, :, :, w0:w0 + kw, :].rearrange(
                            "b i k c -> (b i) k c"))
                    ob = osbp.tile([M, kwin * W], F32, tag="ob")
                    for dw in range(kw):
                        ps = psum.tile([M, W], F32, tag="ps")
                        conv_mms(ps, xt, dw, True, False)
                        conv_mms(ps, xt2, dw, False, True)
                        nc.scalar.activation(
                            ob[:, dw * W:(dw + 1) * W], ps[:],
                            func=mybir.ActivationFunctionType.Sign,
                            bias=sb96[:, 1:2], scale=sb96[:, 0:1])
                    dst = out_ap[n, w0:w0 + kw].rearrange(
                        "k r o c -> (r o) k c")
                    nc.sync.dma_start(
                        dst,
                        ob[:].rearrange("p (k c) -> p k c", c=W)[:, 0:kw, :])

    nc.compile()
    return nc


# ======================= host-side preparation =======================

def host_prep(x, real_weight, gamma, beta, n_cores):
    """Shard + precompute all device inputs. Returns in_maps list."""
    N, Cin, H, W = x.shape
    NW, PADW = _cfg(None, H, W)
    n_img = N // n_cores

    sw = np.sign(real_weight).astype(np.float32)          # [o,i,3,3]
    alpha = np.mean(np.abs(real_weight), axis=(1, 2, 3)).astype(np.float32)

    wjs = np.zeros((3, K, M), np.float32)
    for j in range(3):
        for dh in range(3):
            for r in range(WIN):
                b = r + dh
                wjs[j, b * C:(b + 1) * C, r * C:(r + 1) * C] = sw[:, :, dh, j].T
    wjs = wjs.astype(ml_dtypes.bfloat16)

    selm = np.zeros((M, C), np.float32)
    for r in range(WIN):
        selm[r * C:(r + 1) * C, :] = np.eye(C, dtype=np.float32)

    cvec = np.stack([alpha, alpha * alpha,
                     gamma.astype(np.float32) * alpha,
                     beta.astype(np.float32)], axis=1)    # [16, 4]

    # padded x: rows [-1 .. (NW-1)*6+6], cols [-1 .. W]
    padrows = (NW - 1) * WIN + B - 1 - (H - 1)            # rows past H-1
    in_maps = []
    for c in range(n_cores):
        xs = np.asarray(x[c * n_img:(c + 1) * n_img], np.float32)
        xpad = np.zeros((n_img, Cin, H + 1 + padrows, PADW), np.float32)
        xpad[:, :, 1:H + 1, 1:W + 1] = xs
        hi = xpad.astype(ml_dtypes.bfloat16)
        lo = (xpad - hi.astype(np.float32)).astype(ml_dtypes.bfloat16)

        def prep(a):
            s = a.strides
            v = np.lib.stride_tricks.as_strided(
                a, shape=(n_img, Cin, NW, B, PADW),
                strides=(s[0], s[1], WIN * s[2], s[2], s[3]))
            return np.ascontiguousarray(v.transpose(0, 3, 1, 2, 4))
        in_maps.append({
            "xh": prep(hi), "xl": prep(lo),
            "wj": wjs, "sel": selm, "cvec": cvec,
        })
    return in_maps


def reference_numpy(x, real_weight, gamma, beta):
    """Plain numpy reference (for the small-config sim test)."""
    N, Ci, H, W = x.shape
    Co = real_weight.shape[0]
    alpha = np.mean(np.abs(real_weight), axis=(1, 2, 3))
    w = np.sign(real_weight) * alpha[:, None, None, None]
    xp = np.pad(x, ((0, 0), (0, 0), (1, 1), (1, 1)))
    y = np.zeros((N, Co, H, W), np.float32)
    for kh in range(3):
        for kw in range(3):
            y += np.einsum("oi,nihw->nohw", w[:, :, kh, kw],
                           xp[:, :, kh:kh + H, kw:kw + W])
    mean = y.mean(axis=(0, 2, 3), keepdims=True)
    var = ((y - mean) ** 2).mean(axis=(0, 2, 3), keepdims=True)
    y = (y - mean) / np.sqrt(var + BN_EPS)
    y = y * gamma[None, :, None, None] + beta[None, :, None, None]
    return np.clip(np.sign(y), -1, 1).astype(np.float32)


# ======================= entry point =======================

_NC_CACHE = {}


def _get_nc(key):
    if key not in _NC_CACHE:
        n_img, H, W, n_cores = key
        _NC_CACHE[key] = build_nc(n_img, H, W, n_cores)
    return _NC_CACHE[key]


def kernel(x, real_weight, gamma, beta):
    from concourse.bass_utils import run_bass_kernel_spmd
    x = np.asarray(x)
    n_cores = 8
    n_img = x.shape[0] // n_cores
    H, W = x.shape[2], x.shape[3]
    nc = _get_nc((n_img, H, W, n_cores))
    in_maps = host_prep(np.asarray(x, np.float32),
                        np.asarray(real_weight, np.float32),
                        np.asarray(gamma, np.float32),
                        np.asarray(beta, np.float32), n_cores)
    res = run_bass_kernel_spmd(nc, in_maps, core_ids=list(range(n_cores)))
    out = np.concatenate([unpack_out(res.results[c]["out"], H, W)
                          for c in range(n_cores)], axis=0)
    return out.astype(np.float32)


def unpack_out(dev_out, H, W):
    """[n_img, NW, WIN, C, W] device layout -> [n_img, C, H, W]."""
    n_img = dev_out.shape[0]
    v = dev_out.transpose(0, 3, 1, 2, 4).reshape(n_img, C, -1, W)
    return v[:, :, 0:H, :]
